# revision 37
# baseline (speedup 1.0000x reference)
"""Trainium2 Bass kernel for BEiT-style attention with relative position bias.

Shapes (hardcoded): x (64, 197, 768), 12 heads x 64 dim, rpb table (732, 12).

Sharding: data-parallel over batch -- 8 batches per NeuronCore, weights
replicated (fp16 on device, fp32 accumulation in PSUM).

Per-core dataflow (no on-device transposes):
  qk^T   = W_qk @ x^T          fp16, 4 token chunks of 394, heads pair-packed
  v_nat  = x @ W_v^T           token-major V, per-head ones column memset to 1
  s^T    = k_h^T.T @ q_h^T     keys on partitions; head pairs run concurrently
                               in disjoint PE row groups (tile_position)
  e      = exp(s^T - 5) * exp(rpb^T)   one [128,394] ACT per head, one
                                        [128,788] DVE mul per head pair
  pv     = [v_h | ones].T @ e  rows 0:64 unnormalized out^T, row 64 colsum
  out^T  = pv[0:64] * bcast(1/colsum)
  final  = W_p^T.T @ out^T + (b_p + W_p v_bias)   output-major, fp16 out^T
           (v_bias folded into the proj bias: sum(attn)=1)
"""

import sys

if "/opt/trn_rl_repo" not in sys.path:
    sys.path.insert(0, "/opt/trn_rl_repo")

import numpy as np

import concourse.bass as bass
import concourse.mybir as mybir
import concourse.tile as tile
from concourse import bacc
from concourse.bass_utils import run_bass_kernel_spmd

F32 = mybir.dt.float32
F16 = mybir.dt.float16
AF = mybir.ActivationFunctionType

B, N, C, H, HD = 64, 197, 768, 12, 64
NCORES = 8
BC = B // NCORES          # batches per core (8)
T = BC * N                # tokens per core (1576)
KT = C // 128             # contraction tiles (6)
NCH = 4                   # token chunks for qk
TP = T // NCH             # tokens per chunk (394)
N2 = 2 * N                # 394 (es width: two key tiles)
SCALE = HD ** -0.5
VW = H * (HD + 1)         # v buffer width incl. ones columns (780)
QKP = 1640                # qk buffer cols incl. zero pad for b7 kt2 reads
EXP_SHIFT = -5.0
WARMUP = 8                # always-ready junk matmuls ahead of the first DMAs
WARMUP_LATE = 22          # low-priority junk matmuls: fill PE gaps while DMAs land


def _ktile_layout(w):
    """(768, M) -> (128, 6*M) with k-tile-major columns."""
    m = w.shape[1]
    return np.ascontiguousarray(
        w.reshape(KT, 128, m).transpose(1, 0, 2).reshape(128, KT * m)
    )


def _build_program():
    nc = bacc.Bacc("TRN2", target_bir_lowering=False, debug=False,
                   num_devices=NCORES)

    xt_d = nc.declare_dram_parameter("xt", [128, KT * T], F16, isOutput=False)
    # j-major: per output tile j, its 6 k-slices of 128 cols are contiguous
    wqk_d = nc.declare_dram_parameter("wqk", [128, 12 * C], F16, isOutput=False)
    wv_d = nc.declare_dram_parameter("wv", [128, KT * VW], F16, isOutput=False)
    wp_d = nc.declare_dram_parameter("wp", [128, KT * C], F16, isOutput=False)
    rpbm_d = nc.declare_dram_parameter("rpbm", [128, H * N2], F16, isOutput=False)
    qkb_d = nc.declare_dram_parameter("qkb", [128, KT], F32, isOutput=False)
    pbt_d = nc.declare_dram_parameter("pbt", [128, KT], F32, isOutput=False)
    outT_d = nc.declare_dram_parameter("outT", [128, KT * T], F16, isOutput=True)

    from contextlib import ExitStack

    with tile.TileContext(nc) as tc, ExitStack() as ctx:
        consts = ctx.enter_context(tc.tile_pool(name="consts", bufs=1))
        qk_pool = ctx.enter_context(tc.tile_pool(name="qk", bufs=1))
        v_pool = ctx.enter_context(tc.tile_pool(name="v", bufs=1))
        es_pool = ctx.enter_context(tc.tile_pool(name="es", bufs=2))
        pvs_pool = ctx.enter_context(tc.tile_pool(name="pvs", bufs=2))
        rr_pool = ctx.enter_context(tc.tile_pool(name="rr", bufs=2))
        ot_pool = ctx.enter_context(tc.tile_pool(name="ot", bufs=1))
        fs_pool = ctx.enter_context(tc.tile_pool(name="fs", bufs=14))
        dram_pool = ctx.enter_context(tc.tile_pool(name="dsc", bufs=2, space="DRAM"))
        ps_mm = ctx.enter_context(tc.tile_pool(name="ps_mm", bufs=2, space="PSUM"))
        ps_sc = ctx.enter_context(tc.tile_pool(name="ps_sc", bufs=2, space="PSUM"))
        ps_pv = ctx.enter_context(tc.tile_pool(name="ps_pv", bufs=2, space="PSUM"))

        # ---------------- constant/weight loads (DMA queue is FIFO: put the
        # first compute's dependencies in front)
        wu = consts.tile([128, TP], F16, name="wu")
        nc.vector.memset(wu[:], 0.0)

        wqk_t = [consts.tile([128, C], F16, name=f"wqk{j}") for j in range(12)]
        wv_t = [consts.tile([128, VW], F16, name=f"wv{k}") for k in range(KT)]
        wp_t = [consts.tile([128, C], F16, name=f"wp{k}") for k in range(KT)]
        xt_t = [consts.tile([128, T], F16, name=f"xt{k}") for k in range(KT)]

        qkb = consts.tile([128, KT], F32, name="qkb")
        rpbm = consts.tile([128, H * N2], F16, name="rpbm")
        pbt = consts.tile([128, KT], F32, name="pbt")
        # first qk group needs wqk[0..1] + xt first halves: split the trigger
        # stream across two queues so ~650ns/trigger serialization halves
        HT = T // 2
        QS = [nc.sync, nc.gpsimd, nc.scalar]
        nc.gpsimd.dma_start(qkb[:], qkb_d[:])
        nc.sync.dma_start(wqk_t[0][:], wqk_d[:, 0:C])
        nc.scalar.dma_start(wqk_t[1][:], wqk_d[:, C:2 * C])
        for k in range(KT):
            QS[k % 3].dma_start(xt_t[k][:, 0:HT], xt_d[:, k * T:k * T + HT])
        for j in range(2, 12):
            QS[j % 3].dma_start(wqk_t[j][:], wqk_d[:, j * C:(j + 1) * C])
        for k in range(KT):
            eng = nc.sync if k % 2 == 0 else nc.gpsimd
            eng.dma_start(xt_t[k][:, HT:T], xt_d[:, k * T + HT:(k + 1) * T])
        for k in range(KT):
            eng = nc.sync if k % 2 == 0 else nc.gpsimd
            eng.dma_start(wv_t[k][:], wv_d[:, k * VW:(k + 1) * VW])
        nc.gpsimd.dma_start(rpbm[:], rpbm_d[:])
        for k in range(KT):
            eng = nc.sync if k % 2 == 0 else nc.gpsimd
            eng.dma_start(wp_t[k][:], wp_d[:, k * C:(k + 1) * C])
        nc.sync.dma_start(pbt[:], pbt_d[:])
        nb = consts.tile([128, 1], F32, name="nb")
        nc.vector.memset(nb[:], EXP_SHIFT)

        # persistent buffers
        qkbuf = [qk_pool.tile([128, QKP], F16, name=f"qkb{j}") for j in range(12)]
        vbuf = {}
        for p in range(4):
            for t in range(4):
                vbuf[(p, t)] = v_pool.tile([128, VW], F16, name=f"v{p}{t}")
        ot = [ot_pool.tile([128, T], F16, name=f"ot{k}") for k in range(KT)]

        # ---------------- PE warmup: junk matmuls with no DMA deps keep the
        # HAM activity window busy while the first weights stream in. The
        # accumulated result is written into qkbuf[6]'s zero pad (live data:
        # read by batch-7 kt2 scores; values are 0*0 = 0).
        pw = ps_mm.tile([128, TP], F32, tag="mm", name="pwu")
        for i in range(WARMUP):
            nc.tensor.matmul(pw[:], wu[:, 0:128], wu[:],
                             start=(i == 0), stop=(i == WARMUP - 1))
        nc.scalar.copy(qkbuf[6][:, T:QKP], pw[:, 0:QKP - T])

        def emit_warmup_late():
            pw2 = ps_mm.tile([128, TP], F32, tag="mm", name="pwu2")
            for i in range(WARMUP_LATE):
                nc.tensor.matmul(pw2[:], wu[:, 0:128], wu[:],
                                 start=(i == 0), stop=(i == WARMUP_LATE - 1))
            nc.scalar.copy(qkbuf[6][:, T:QKP], pw2[:, 0:QKP - T])

        for j in range(7, 12):
            nc.vector.memset(qkbuf[j][:, T:QKP], 0.0)

        def emit_warmup_mid():
            # fills the DMA-stagger gaps inside qk chunk 0: the sc pool's
            # slots are free until batch 0, so these junk MMs schedule
            # immediately (the mm pool's slots are held by blocked qk groups)
            pw3 = ps_sc.tile([128, 1024], F32, tag="sc", name="pwm")
            for i in range(14):
                nc.tensor.matmul(pw3[:, 0:TP], wu[:, 0:128], wu[:],
                                 start=(i == 0), stop=(i == 13))
            nc.scalar.copy(qkbuf[6][:, T:QKP], pw3[:, 0:QKP - T])

        # ---------------- emit helpers
        QKCH = [(0, 512), (512, 512), (1024, 512), (1536, 40)]

        def emit_qk_chunk(c):
            co, cw = QKCH[c]
            for j in range(12):
                pq = ps_mm.tile([128, 512], F32, tag="mm")
                for i in range(KT):
                    k = (j + i) % KT
                    nc.tensor.matmul(
                        pq[:, 0:cw],
                        wqk_t[j][:, k * 128:(k + 1) * 128],
                        xt_t[k][:, co:co + cw],
                        start=(i == 0), stop=(i == KT - 1),
                    )
                if c < 2:
                    if j < 6:
                        nc.scalar.activation(
                            qkbuf[j][:, co:co + cw], pq[:, 0:cw],
                            AF.Identity, bias=qkb[:, j:j + 1])
                    else:
                        nc.scalar.copy(qkbuf[j][:, co:co + cw], pq[:, 0:cw])
                elif j < 6:
                    nc.vector.tensor_scalar_add(
                        qkbuf[j][:, co:co + cw], pq[:, 0:cw], qkb[:, j:j + 1])
                else:
                    nc.vector.tensor_copy(qkbuf[j][:, co:co + cw], pq[:, 0:cw])

        def emit_v_batch(b):
            p, half = divmod(b, 2)
            for t, (toff, rows) in enumerate(
                    [(half * N, 128), (half * N + 128, 69)]):
                vt = vbuf[(p, half * 2 + t)]
                goff = p * 2 * N + toff
                for ho, hw_ in ((0, 512), (512, VW - 512)):
                    pv = ps_mm.tile([128, 512], F32, tag="mm")
                    for k in range(KT):
                        nc.tensor.matmul(
                            pv[0:rows, 0:hw_],
                            xt_t[k][:, goff:goff + rows],
                            wv_t[k][:, ho:ho + hw_],
                            start=(k == 0), stop=(k == KT - 1),
                        )
                    if b < 3:
                        nc.scalar.copy(vt[0:rows, ho:ho + hw_],
                                       pv[0:rows, 0:hw_])
                    else:
                        nc.vector.tensor_copy(
                            vt[0:rows, ho:ho + hw_], pv[0:rows, 0:hw_])
                # ones columns (65th col of each head) for the colsum row
                _v = vt[:]
                ones_ap = bass.AP(tensor=_v.tensor, offset=_v.offset + HD,
                                  ap=[list(_v.ap[0])] + [[HD + 1, H]])
                nc.vector.memset(ones_ap, 1.0)

        def emit_batch(b, pvs_dve=False):
            pair, half = divmod(b, 2)
            boff = b * N
            es_tiles = []
            # scores: head pairs share the PE via disjoint row groups
            for jt in range(6):
                kt_ = qkbuf[6 + jt]
                qt_ = qkbuf[jt]
                psp = ps_sc.tile([128, 1024], F32, tag="sc")
                for kt2 in range(2):
                    ko = boff + kt2 * 128
                    for h2 in range(2):
                        hb = h2 * 64
                        nc.tensor.matmul(
                            psp[:, h2 * 512 + kt2 * N:h2 * 512 + kt2 * N + N],
                            kt_[hb:hb + 64, ko:ko + 128],
                            qt_[hb:hb + 64, boff:boff + N],
                            start=True, stop=True,
                            tile_position=(hb, 0),
                        )
                if jt % 3 == 0:
                    es3 = es_pool.tile([128, 6 * N2], F16, tag=f"es{jt // 3}")
                    es_tiles.append(es3)
                eo = (jt % 3) * 2 * N2
                # one exp for both heads: strided AP skips psum cols 394:512
                _pi = psp[:]
                _eo = es3[:]
                nc.scalar.activation(
                    bass.AP(tensor=_eo.tensor, offset=_eo.offset + eo,
                            ap=[list(_eo.ap[0])] + [[N2, 2], [1, N2]]),
                    bass.AP(tensor=_pi.tensor, offset=_pi.offset,
                            ap=[list(_pi.ap[0])] + [[512, 2], [1, N2]]),
                    AF.Exp, bias=nb[:])
                if jt % 3 == 2:
                    g = jt // 3
                    nc.vector.tensor_mul(
                        es3[:], es3[:],
                        rpbm[:, g * 6 * N2:(g + 1) * 6 * N2])

            # pv + evacuation (2 heads per PSUM bank)
            pvs = pvs_pool.tile([65, H * N], F16, tag="pvs")
            v0 = vbuf[(pair, half * 2)]
            v1 = vbuf[(pair, half * 2 + 1)]
            for jt in range(6):
                ppv = ps_pv.tile([65, N2], F32, tag="pv")
                es = es_tiles[jt // 3]
                for h2 in range(2):
                    h = 2 * jt + h2
                    eo = (jt % 3) * 2 * N2 + h2 * N2
                    nc.tensor.matmul(
                        ppv[:, h2 * N:h2 * N + N],
                        v0[0:128, h * 65:(h + 1) * 65],
                        es[:, eo:eo + N], start=True, stop=False,
                    )
                    nc.tensor.matmul(
                        ppv[:, h2 * N:h2 * N + N],
                        v1[0:69, h * 65:(h + 1) * 65],
                        es[0:69, eo + N:eo + N2], start=False, stop=True,
                    )
                if pvs_dve and jt % 2 == 0:
                    nc.vector.tensor_copy(pvs[:, jt * N2:(jt + 1) * N2], ppv[:])
                else:
                    nc.scalar.copy(pvs[:, jt * N2:(jt + 1) * N2], ppv[:])

            # colsum -> reciprocal -> broadcast (DRAM hop for the bcast)
            rsb = rr_pool.tile([H, N], F16, tag="rsb")
            nc.gpsimd.dma_start(rsb[:], pvs[64:65, :])
            rsb32 = rr_pool.tile([H, N], F32, tag="rsb32")
            nc.vector.tensor_copy(rsb32[:], rsb[:])
            rsr32 = rr_pool.tile([H, N], F32, tag="rsr32")
            nc.vector.reciprocal_approx_fast(rsr32[:], rsb32[:])
            rsr = rr_pool.tile([H, N], F16, tag="rsr")
            nc.vector.tensor_copy(rsr[:], rsr32[:])
            dsc2 = dram_pool.tile([1, H * N], F16, tag="dsc2")
            nc.gpsimd.dma_start(dsc2[0:1, :], rsr[:])
            rb = rr_pool.tile([64, H * N], F16, tag="rb")
            _d2 = dsc2[:]
            nc.gpsimd.dma_start(
                rb[:],
                bass.AP(tensor=_d2.tensor, offset=_d2.offset,
                        ap=[[0, 64], [1, H * N]]))

            # normalize into out^T (proj's moving operand layout)
            for h in range(12):
                jt, h2 = divmod(h, 2)
                hb = h2 * 64
                nc.vector.tensor_mul(
                    ot[jt][hb:hb + 64, boff:boff + N],
                    pvs[0:64, jt * N2 + h2 * N:jt * N2 + h2 * N + N],
                    rb[0:64, h * N:(h + 1) * N],
                )

        def emit_proj_span(po, pw_, dve=False):
            for jt in range(6):
                pf = ps_mm.tile([128, 512], F32, tag="mm")
                for i in range(KT):
                    k = (jt + i) % KT
                    nc.tensor.matmul(
                        pf[:, 0:pw_],
                        wp_t[k][:, jt * 128:(jt + 1) * 128],
                        ot[k][:, po:po + pw_],
                        start=(i == 0), stop=(i == KT - 1),
                    )
                fs = fs_pool.tile([128, N2], F16, tag="fs", name="fs")
                if dve:
                    nc.vector.tensor_scalar_add(fs[:, 0:pw_], pf[:, 0:pw_],
                                                pbt[:, jt:jt + 1])
                else:
                    nc.scalar.activation(fs[:, 0:pw_], pf[:, 0:pw_],
                                         AF.Identity, bias=pbt[:, jt:jt + 1])
                nc.sync.dma_start(
                    outT_d[:, jt * T + po:jt * T + po + pw_], fs[:, 0:pw_])

        def emit_proj_pair(p, dve=False):
            emit_proj_span(p * N2, N2, dve)

        # ---------------- schedule (priority = emission order; the Tile
        # scheduler fills engine gaps with ready lower-priority work)
        # 512-col qk chunks; batch b reads k columns up to b*197+256, so:
        # b0,b1 <- chunk0; b2,b3 <- chunk1; b4,b5 <- chunk2; b6 <- chunk2;
        # b7 <- chunk3 + pad
        emit_warmup_late()
        emit_qk_chunk(0)
        emit_warmup_mid()
        emit_v_batch(0)
        emit_batch(0)
        emit_v_batch(1)
        emit_batch(1)
        emit_qk_chunk(1)
        emit_v_batch(2)
        emit_batch(2)
        emit_v_batch(3)
        emit_batch(3)
        emit_qk_chunk(2)
        emit_proj_pair(0)
        emit_v_batch(4)
        emit_batch(4)
        emit_qk_chunk(3)
        emit_v_batch(5)
        emit_batch(5, pvs_dve=True)
        emit_proj_pair(1)
        emit_v_batch(6)
        emit_batch(6, pvs_dve=True)
        emit_proj_pair(2, dve=True)
        emit_v_batch(7)
        emit_batch(7, pvs_dve=True)
        emit_proj_span(6 * N, N, dve=True)
        emit_proj_span(7 * N, N, dve=True)

    nc.compile()
    return nc


_PROGRAM_CACHE = {}


def _get_program():
    if "nc" not in _PROGRAM_CACHE:
        _PROGRAM_CACHE["nc"] = _build_program()
    return _PROGRAM_CACHE["nc"]


def _host_prep(x, qkv_w, q_bias, v_bias, rpb_table, proj_w, proj_b,
               rel_pos_index):
    x = np.asarray(x, dtype=np.float32)
    qkv_w = np.asarray(qkv_w, dtype=np.float32)
    q_bias = np.asarray(q_bias, dtype=np.float32)
    v_bias = np.asarray(v_bias, dtype=np.float32)
    rpb_table = np.asarray(rpb_table, dtype=np.float32)
    proj_w = np.asarray(proj_w, dtype=np.float32)
    proj_b = np.asarray(proj_b, dtype=np.float32)
    rel_pos_index = np.asarray(rel_pos_index)

    w_q, w_k, w_v = qkv_w[0:C], qkv_w[C:2 * C], qkv_w[2 * C:3 * C]

    # qk^T weights: q columns pre-scaled; j-major layout [128, 12*768]:
    # output tile j's 6 k-slices of 128 cols contiguous
    w_qkT = np.concatenate([w_q.T * SCALE, w_k.T], axis=1)  # (768, 1536)
    wqk_kt = _ktile_layout(w_qkT)  # [128, k*1536 + j*128]
    wqk_dev = np.ascontiguousarray(
        wqk_kt.reshape(128, KT, 12, 128).transpose(0, 2, 1, 3)
        .reshape(128, 12 * C)).astype(np.float16)

    qkb = np.ascontiguousarray(
        (q_bias * SCALE).reshape(KT, 128).T).astype(np.float32)

    # v weights with a zero column after each head's 64 (ones come from a
    # device-side memset)
    w_vT_pad = np.zeros((C, VW), dtype=np.float32)
    for h in range(H):
        w_vT_pad[:, h * 65:h * 65 + 64] = w_v.T[:, h * 64:(h + 1) * 64]
    wv_dev = _ktile_layout(w_vT_pad).astype(np.float16)

    wp_dev = _ktile_layout(np.ascontiguousarray(proj_w.T)).astype(np.float16)
    # v_bias folded into the proj bias (attention rows sum to 1)
    pbt = np.ascontiguousarray(
        (proj_b + proj_w @ v_bias).reshape(KT, 128).T).astype(np.float32)

    # exp(rpb^T): [key, query, head] -> merged per-head [128, 394] blocks
    rpb_g = rpb_table[rel_pos_index.reshape(-1)].reshape(N, N, H)
    erT = np.exp(rpb_g.transpose(1, 0, 2))  # (key, query, head)
    rpbm = np.zeros((128, H * N2), dtype=np.float16)
    for h in range(H):
        rpbm[:, h * N2:h * N2 + N] = erT[0:128, :, h]
        rpbm[0:69, h * N2 + N:(h + 1) * N2] = erT[128:N, :, h]
    rpbm = np.ascontiguousarray(rpbm)

    shared = {
        "wqk": wqk_dev, "wv": wv_dev, "wp": wp_dev,
        "rpbm": rpbm, "qkb": qkb, "pbt": pbt,
    }

    in_maps = []
    for c in range(NCORES):
        xc = x[c * BC:(c + 1) * BC].reshape(T, C)
        xt_dev = _ktile_layout(np.ascontiguousarray(xc.T).reshape(C, T)
                               ).astype(np.float16)
        in_maps.append({"xt": xt_dev, **shared})
    return in_maps


def _ensure_devices():
    import jax

    try:
        if len(jax.devices()) >= NCORES:
            return
    except Exception:
        pass
    try:
        jax.config.update("jax_platforms", "axon")
    except Exception:
        pass


def kernel(x, qkv_w, q_bias, v_bias, rpb_table, proj_w, proj_b,
           rel_pos_index, _trace=False, _trace_kwargs=None):
    _ensure_devices()
    nc = _get_program()
    in_maps = _host_prep(x, qkv_w, q_bias, v_bias, rpb_table, proj_w, proj_b,
                         rel_pos_index)
    res = run_bass_kernel_spmd(
        nc, in_maps, core_ids=list(range(NCORES)),
        trace=_trace, **(_trace_kwargs or {}),
    )
    outs = []
    for c in range(NCORES):
        oT = res.results[c]["outT"]  # [128, KT*T] fp16
        o = oT.reshape(128, KT, T).transpose(1, 0, 2).reshape(C, T)
        outs.append(np.ascontiguousarray(o.T).reshape(BC, N, C))
    out = np.concatenate(outs, axis=0).astype(np.float32)
    if _trace:
        kernel._last_results = res
    return out


# revision 38
# speedup vs baseline: 1.0152x; 1.0152x over previous
"""Trainium2 Bass kernel for BEiT-style attention with relative position bias.

Shapes (hardcoded): x (64, 197, 768), 12 heads x 64 dim, rpb table (732, 12).

Sharding: data-parallel over batch -- 8 batches per NeuronCore, weights
replicated (fp16 on device, fp32 accumulation in PSUM).

Per-core dataflow (no on-device transposes):
  qk^T   = W_qk @ x^T          fp16, 4 token chunks of 394, heads pair-packed
  v_nat  = x @ W_v^T           token-major V, per-head ones column memset to 1
  s^T    = k_h^T.T @ q_h^T     keys on partitions; head pairs run concurrently
                               in disjoint PE row groups (tile_position)
  e      = exp(s^T - 5) * exp(rpb^T)   one [128,394] ACT per head, one
                                        [128,788] DVE mul per head pair
  pv     = [v_h | ones].T @ e  rows 0:64 unnormalized out^T, row 64 colsum
  out^T  = pv[0:64] * bcast(1/colsum)
  final  = W_p^T.T @ out^T + (b_p + W_p v_bias)   output-major, fp16 out^T
           (v_bias folded into the proj bias: sum(attn)=1)
"""

import sys

if "/opt/trn_rl_repo" not in sys.path:
    sys.path.insert(0, "/opt/trn_rl_repo")

import numpy as np

import concourse.bass as bass
import concourse.mybir as mybir
import concourse.tile as tile
from concourse import bacc
from concourse.bass_utils import run_bass_kernel_spmd

F32 = mybir.dt.float32
F16 = mybir.dt.float16
AF = mybir.ActivationFunctionType

B, N, C, H, HD = 64, 197, 768, 12, 64
NCORES = 8
BC = B // NCORES          # batches per core (8)
T = BC * N                # tokens per core (1576)
KT = C // 128             # contraction tiles (6)
NCH = 4                   # token chunks for qk
TP = T // NCH             # tokens per chunk (394)
N2 = 2 * N                # 394 (es width: two key tiles)
SCALE = HD ** -0.5
VW = H * (HD + 1)         # v buffer width incl. ones columns (780)
QKP = 1640                # qk buffer cols incl. zero pad for b7 kt2 reads
EXP_SHIFT = -5.0
WARMUP = 8                # always-ready junk matmuls ahead of the first DMAs
WARMUP_LATE = 22          # low-priority junk matmuls: fill PE gaps while DMAs land


def _ktile_layout(w):
    """(768, M) -> (128, 6*M) with k-tile-major columns."""
    m = w.shape[1]
    return np.ascontiguousarray(
        w.reshape(KT, 128, m).transpose(1, 0, 2).reshape(128, KT * m)
    )


def _build_program():
    nc = bacc.Bacc("TRN2", target_bir_lowering=False, debug=False,
                   num_devices=NCORES)

    xt_d = nc.declare_dram_parameter("xt", [128, KT * T], F16, isOutput=False)
    # j-major: per output tile j, its 6 k-slices of 128 cols are contiguous
    wqk_d = nc.declare_dram_parameter("wqk", [128, 12 * C], F16, isOutput=False)
    wv_d = nc.declare_dram_parameter("wv", [128, KT * VW], F16, isOutput=False)
    wp_d = nc.declare_dram_parameter("wp", [128, KT * C], F16, isOutput=False)
    rpbm_d = nc.declare_dram_parameter("rpbm", [128, H * N2], F16, isOutput=False)
    qkb_d = nc.declare_dram_parameter("qkb", [128, KT], F32, isOutput=False)
    pbt_d = nc.declare_dram_parameter("pbt", [128, KT], F32, isOutput=False)
    outT_d = nc.declare_dram_parameter("outT", [128, KT * T], F16, isOutput=True)

    from contextlib import ExitStack

    with tile.TileContext(nc) as tc, ExitStack() as ctx:
        consts = ctx.enter_context(tc.tile_pool(name="consts", bufs=1))
        qk_pool = ctx.enter_context(tc.tile_pool(name="qk", bufs=1))
        v_pool = ctx.enter_context(tc.tile_pool(name="v", bufs=1))
        es_pool = ctx.enter_context(tc.tile_pool(name="es", bufs=2))
        pvs_pool = ctx.enter_context(tc.tile_pool(name="pvs", bufs=2))
        rr_pool = ctx.enter_context(tc.tile_pool(name="rr", bufs=2))
        ot_pool = ctx.enter_context(tc.tile_pool(name="ot", bufs=1))
        fs_pool = ctx.enter_context(tc.tile_pool(name="fs", bufs=14))
        dram_pool = ctx.enter_context(tc.tile_pool(name="dsc", bufs=2, space="DRAM"))
        ps_mm = ctx.enter_context(tc.tile_pool(name="ps_mm", bufs=2, space="PSUM"))
        ps_sc = ctx.enter_context(tc.tile_pool(name="ps_sc", bufs=2, space="PSUM"))
        ps_pv = ctx.enter_context(tc.tile_pool(name="ps_pv", bufs=2, space="PSUM"))

        # ---------------- constant/weight loads (DMA queue is FIFO: put the
        # first compute's dependencies in front)
        wu = consts.tile([128, TP], F16, name="wu")
        nc.vector.memset(wu[:], 0.0)

        wqk_t = [consts.tile([128, C], F16, name=f"wqk{j}") for j in range(12)]
        wv_t = [consts.tile([128, VW], F16, name=f"wv{k}") for k in range(KT)]
        wp_t = [consts.tile([128, C], F16, name=f"wp{k}") for k in range(KT)]
        xt_t = [consts.tile([128, T], F16, name=f"xt{k}") for k in range(KT)]

        qkb = consts.tile([128, KT], F32, name="qkb")
        rpbm = consts.tile([128, H * N2], F16, name="rpbm")
        pbt = consts.tile([128, KT], F32, name="pbt")
        # first qk group needs wqk[0..1] + xt first halves: split the trigger
        # stream across two queues so ~650ns/trigger serialization halves
        HT = T // 2
        QS = [nc.sync, nc.gpsimd, nc.scalar]
        nc.gpsimd.dma_start(qkb[:], qkb_d[:])
        nc.sync.dma_start(wqk_t[0][:], wqk_d[:, 0:C])
        nc.scalar.dma_start(wqk_t[1][:], wqk_d[:, C:2 * C])
        for k in range(KT):
            QS[k % 3].dma_start(xt_t[k][:, 0:HT], xt_d[:, k * T:k * T + HT])
        for j in range(2, 12):
            QS[j % 3].dma_start(wqk_t[j][:], wqk_d[:, j * C:(j + 1) * C])
        for k in range(KT):
            eng = nc.sync if k % 2 == 0 else nc.gpsimd
            eng.dma_start(xt_t[k][:, HT:T], xt_d[:, k * T + HT:(k + 1) * T])
        for k in range(KT):
            eng = nc.sync if k % 2 == 0 else nc.gpsimd
            eng.dma_start(wv_t[k][:], wv_d[:, k * VW:(k + 1) * VW])
        nc.gpsimd.dma_start(rpbm[:], rpbm_d[:])
        for k in range(KT):
            eng = nc.sync if k % 2 == 0 else nc.gpsimd
            eng.dma_start(wp_t[k][:], wp_d[:, k * C:(k + 1) * C])
        nc.sync.dma_start(pbt[:], pbt_d[:])
        nb = consts.tile([128, 1], F32, name="nb")
        nc.vector.memset(nb[:], EXP_SHIFT)

        # persistent buffers
        qkbuf = [qk_pool.tile([128, QKP], F16, name=f"qkb{j}") for j in range(12)]
        vbuf = {}
        for p in range(4):
            for t in range(4):
                vbuf[(p, t)] = v_pool.tile([128, VW], F16, name=f"v{p}{t}")
        ot = [ot_pool.tile([128, T], F16, name=f"ot{k}") for k in range(KT)]

        # ---------------- PE warmup: junk matmuls with no DMA deps keep the
        # HAM activity window busy while the first weights stream in. The
        # accumulated result is written into qkbuf[6]'s zero pad (live data:
        # read by batch-7 kt2 scores; values are 0*0 = 0).
        pw = ps_mm.tile([128, TP], F32, tag="mm", name="pwu")
        for i in range(WARMUP):
            nc.tensor.matmul(pw[:], wu[:, 0:128], wu[:],
                             start=(i == 0), stop=(i == WARMUP - 1))
        nc.scalar.copy(qkbuf[6][:, T:QKP], pw[:, 0:QKP - T])

        def emit_warmup_late():
            pw2 = ps_mm.tile([128, TP], F32, tag="mm", name="pwu2")
            for i in range(WARMUP_LATE):
                nc.tensor.matmul(pw2[:], wu[:, 0:128], wu[:],
                                 start=(i == 0), stop=(i == WARMUP_LATE - 1))
            nc.scalar.copy(qkbuf[6][:, T:QKP], pw2[:, 0:QKP - T])

        for j in range(7, 12):
            nc.vector.memset(qkbuf[j][:, T:QKP], 0.0)

        # ---------------- emit helpers
        QKCH = [(0, 512), (512, 512), (1024, 512), (1536, 40)]

        def emit_qk_chunk(c):
            co, cw = QKCH[c]
            for j in range(12):
                pq = ps_mm.tile([128, 512], F32, tag="mm")
                for i in range(KT):
                    k = (j + i) % KT
                    nc.tensor.matmul(
                        pq[:, 0:cw],
                        wqk_t[j][:, k * 128:(k + 1) * 128],
                        xt_t[k][:, co:co + cw],
                        start=(i == 0), stop=(i == KT - 1),
                    )
                if c < 2:
                    if j < 6:
                        nc.scalar.activation(
                            qkbuf[j][:, co:co + cw], pq[:, 0:cw],
                            AF.Identity, bias=qkb[:, j:j + 1])
                    else:
                        nc.scalar.copy(qkbuf[j][:, co:co + cw], pq[:, 0:cw])
                elif j < 6:
                    nc.vector.tensor_scalar_add(
                        qkbuf[j][:, co:co + cw], pq[:, 0:cw], qkb[:, j:j + 1])
                else:
                    nc.vector.tensor_copy(qkbuf[j][:, co:co + cw], pq[:, 0:cw])

        def emit_v_batch(b):
            p, half = divmod(b, 2)
            for t, (toff, rows) in enumerate(
                    [(half * N, 128), (half * N + 128, 69)]):
                vt = vbuf[(p, half * 2 + t)]
                goff = p * 2 * N + toff
                for ho, hw_ in ((0, 512), (512, VW - 512)):
                    pv = ps_mm.tile([128, 512], F32, tag="mm")
                    for k in range(KT):
                        nc.tensor.matmul(
                            pv[0:rows, 0:hw_],
                            xt_t[k][:, goff:goff + rows],
                            wv_t[k][:, ho:ho + hw_],
                            start=(k == 0), stop=(k == KT - 1),
                        )
                    if b < 3:
                        nc.scalar.copy(vt[0:rows, ho:ho + hw_],
                                       pv[0:rows, 0:hw_])
                    else:
                        nc.vector.tensor_copy(
                            vt[0:rows, ho:ho + hw_], pv[0:rows, 0:hw_])
                # ones columns (65th col of each head) for the colsum row
                _v = vt[:]
                ones_ap = bass.AP(tensor=_v.tensor, offset=_v.offset + HD,
                                  ap=[list(_v.ap[0])] + [[HD + 1, H]])
                nc.vector.memset(ones_ap, 1.0)

        def emit_batch(b, pvs_dve=False):
            pair, half = divmod(b, 2)
            boff = b * N
            es_tiles = []
            # scores: head pairs share the PE via disjoint row groups
            for jt in range(6):
                kt_ = qkbuf[6 + jt]
                qt_ = qkbuf[jt]
                psp = ps_sc.tile([128, 1024], F32, tag="sc")
                for kt2 in range(2):
                    ko = boff + kt2 * 128
                    for h2 in range(2):
                        hb = h2 * 64
                        nc.tensor.matmul(
                            psp[:, h2 * 512 + kt2 * N:h2 * 512 + kt2 * N + N],
                            kt_[hb:hb + 64, ko:ko + 128],
                            qt_[hb:hb + 64, boff:boff + N],
                            start=True, stop=True,
                            tile_position=(hb, 0),
                        )
                if jt % 3 == 0:
                    es3 = es_pool.tile([128, 6 * N2], F16, tag=f"es{jt // 3}")
                    es_tiles.append(es3)
                eo = (jt % 3) * 2 * N2
                # one exp for both heads: strided AP skips psum cols 394:512
                _pi = psp[:]
                _eo = es3[:]
                nc.scalar.activation(
                    bass.AP(tensor=_eo.tensor, offset=_eo.offset + eo,
                            ap=[list(_eo.ap[0])] + [[N2, 2], [1, N2]]),
                    bass.AP(tensor=_pi.tensor, offset=_pi.offset,
                            ap=[list(_pi.ap[0])] + [[512, 2], [1, N2]]),
                    AF.Exp, bias=nb[:])
                if jt % 3 == 2:
                    g = jt // 3
                    nc.vector.tensor_mul(
                        es3[:], es3[:],
                        rpbm[:, g * 6 * N2:(g + 1) * 6 * N2])

            # pv + evacuation (2 heads per PSUM bank)
            pvs = pvs_pool.tile([65, H * N], F16, tag="pvs")
            v0 = vbuf[(pair, half * 2)]
            v1 = vbuf[(pair, half * 2 + 1)]
            for jt in range(6):
                ppv = ps_pv.tile([65, N2], F32, tag="pv")
                es = es_tiles[jt // 3]
                for h2 in range(2):
                    h = 2 * jt + h2
                    eo = (jt % 3) * 2 * N2 + h2 * N2
                    nc.tensor.matmul(
                        ppv[:, h2 * N:h2 * N + N],
                        v0[0:128, h * 65:(h + 1) * 65],
                        es[:, eo:eo + N], start=True, stop=False,
                    )
                    nc.tensor.matmul(
                        ppv[:, h2 * N:h2 * N + N],
                        v1[0:69, h * 65:(h + 1) * 65],
                        es[0:69, eo + N:eo + N2], start=False, stop=True,
                    )
                if pvs_dve and jt % 2 == 0:
                    nc.vector.tensor_copy(pvs[:, jt * N2:(jt + 1) * N2], ppv[:])
                else:
                    nc.scalar.copy(pvs[:, jt * N2:(jt + 1) * N2], ppv[:])

            # colsum -> reciprocal -> broadcast (DRAM hop for the bcast)
            rsb = rr_pool.tile([H, N], F16, tag="rsb")
            nc.gpsimd.dma_start(rsb[:], pvs[64:65, :])
            rsb32 = rr_pool.tile([H, N], F32, tag="rsb32")
            nc.vector.tensor_copy(rsb32[:], rsb[:])
            rsr32 = rr_pool.tile([H, N], F32, tag="rsr32")
            nc.vector.reciprocal_approx_fast(rsr32[:], rsb32[:])
            rsr = rr_pool.tile([H, N], F16, tag="rsr")
            nc.vector.tensor_copy(rsr[:], rsr32[:])
            dsc2 = dram_pool.tile([1, H * N], F16, tag="dsc2")
            nc.gpsimd.dma_start(dsc2[0:1, :], rsr[:])
            rb = rr_pool.tile([64, H * N], F16, tag="rb")
            _d2 = dsc2[:]
            nc.gpsimd.dma_start(
                rb[:],
                bass.AP(tensor=_d2.tensor, offset=_d2.offset,
                        ap=[[0, 64], [1, H * N]]))

            # normalize into out^T (proj's moving operand layout)
            for h in range(12):
                jt, h2 = divmod(h, 2)
                hb = h2 * 64
                nc.vector.tensor_mul(
                    ot[jt][hb:hb + 64, boff:boff + N],
                    pvs[0:64, jt * N2 + h2 * N:jt * N2 + h2 * N + N],
                    rb[0:64, h * N:(h + 1) * N],
                )

        def emit_proj_span(po, pw_, dve=False):
            for jt in range(6):
                pf = ps_mm.tile([128, 512], F32, tag="mm")
                for i in range(KT):
                    k = (jt + i) % KT
                    nc.tensor.matmul(
                        pf[:, 0:pw_],
                        wp_t[k][:, jt * 128:(jt + 1) * 128],
                        ot[k][:, po:po + pw_],
                        start=(i == 0), stop=(i == KT - 1),
                    )
                fs = fs_pool.tile([128, N2], F16, tag="fs", name="fs")
                if dve:
                    nc.vector.tensor_scalar_add(fs[:, 0:pw_], pf[:, 0:pw_],
                                                pbt[:, jt:jt + 1])
                else:
                    nc.scalar.activation(fs[:, 0:pw_], pf[:, 0:pw_],
                                         AF.Identity, bias=pbt[:, jt:jt + 1])
                nc.sync.dma_start(
                    outT_d[:, jt * T + po:jt * T + po + pw_], fs[:, 0:pw_])

        def emit_proj_pair(p, dve=False):
            emit_proj_span(p * N2, N2, dve)

        # ---------------- schedule (priority = emission order; the Tile
        # scheduler fills engine gaps with ready lower-priority work)
        # 512-col qk chunks; batch b reads k columns up to b*197+256, so:
        # b0,b1 <- chunk0; b2,b3 <- chunk1; b4,b5 <- chunk2; b6 <- chunk2;
        # b7 <- chunk3 + pad
        emit_warmup_late()
        emit_qk_chunk(0)
        emit_v_batch(0)
        emit_batch(0)
        emit_v_batch(1)
        emit_batch(1)
        emit_qk_chunk(1)
        emit_v_batch(2)
        emit_batch(2)
        emit_v_batch(3)
        emit_batch(3)
        emit_qk_chunk(2)
        emit_proj_pair(0)
        emit_v_batch(4)
        emit_batch(4)
        emit_qk_chunk(3)
        emit_v_batch(5)
        emit_batch(5, pvs_dve=True)
        emit_proj_pair(1)
        emit_v_batch(6)
        emit_batch(6, pvs_dve=True)
        emit_proj_pair(2, dve=True)
        emit_v_batch(7)
        emit_batch(7, pvs_dve=True)
        emit_proj_span(6 * N, N, dve=True)
        emit_proj_span(7 * N, N, dve=True)

    nc.compile()
    return nc


_PROGRAM_CACHE = {}


def _get_program():
    if "nc" not in _PROGRAM_CACHE:
        _PROGRAM_CACHE["nc"] = _build_program()
    return _PROGRAM_CACHE["nc"]


def _host_prep(x, qkv_w, q_bias, v_bias, rpb_table, proj_w, proj_b,
               rel_pos_index):
    x = np.asarray(x, dtype=np.float32)
    qkv_w = np.asarray(qkv_w, dtype=np.float32)
    q_bias = np.asarray(q_bias, dtype=np.float32)
    v_bias = np.asarray(v_bias, dtype=np.float32)
    rpb_table = np.asarray(rpb_table, dtype=np.float32)
    proj_w = np.asarray(proj_w, dtype=np.float32)
    proj_b = np.asarray(proj_b, dtype=np.float32)
    rel_pos_index = np.asarray(rel_pos_index)

    w_q, w_k, w_v = qkv_w[0:C], qkv_w[C:2 * C], qkv_w[2 * C:3 * C]

    # qk^T weights: q columns pre-scaled; j-major layout [128, 12*768]:
    # output tile j's 6 k-slices of 128 cols contiguous
    w_qkT = np.concatenate([w_q.T * SCALE, w_k.T], axis=1)  # (768, 1536)
    wqk_kt = _ktile_layout(w_qkT)  # [128, k*1536 + j*128]
    wqk_dev = np.ascontiguousarray(
        wqk_kt.reshape(128, KT, 12, 128).transpose(0, 2, 1, 3)
        .reshape(128, 12 * C)).astype(np.float16)

    qkb = np.ascontiguousarray(
        (q_bias * SCALE).reshape(KT, 128).T).astype(np.float32)

    # v weights with a zero column after each head's 64 (ones come from a
    # device-side memset)
    w_vT_pad = np.zeros((C, VW), dtype=np.float32)
    for h in range(H):
        w_vT_pad[:, h * 65:h * 65 + 64] = w_v.T[:, h * 64:(h + 1) * 64]
    wv_dev = _ktile_layout(w_vT_pad).astype(np.float16)

    wp_dev = _ktile_layout(np.ascontiguousarray(proj_w.T)).astype(np.float16)
    # v_bias folded into the proj bias (attention rows sum to 1)
    pbt = np.ascontiguousarray(
        (proj_b + proj_w @ v_bias).reshape(KT, 128).T).astype(np.float32)

    # exp(rpb^T): [key, query, head] -> merged per-head [128, 394] blocks
    rpb_g = rpb_table[rel_pos_index.reshape(-1)].reshape(N, N, H)
    erT = np.exp(rpb_g.transpose(1, 0, 2))  # (key, query, head)
    rpbm = np.zeros((128, H * N2), dtype=np.float16)
    for h in range(H):
        rpbm[:, h * N2:h * N2 + N] = erT[0:128, :, h]
        rpbm[0:69, h * N2 + N:(h + 1) * N2] = erT[128:N, :, h]
    rpbm = np.ascontiguousarray(rpbm)

    shared = {
        "wqk": wqk_dev, "wv": wv_dev, "wp": wp_dev,
        "rpbm": rpbm, "qkb": qkb, "pbt": pbt,
    }

    in_maps = []
    for c in range(NCORES):
        xc = x[c * BC:(c + 1) * BC].reshape(T, C)
        xt_dev = _ktile_layout(np.ascontiguousarray(xc.T).reshape(C, T)
                               ).astype(np.float16)
        in_maps.append({"xt": xt_dev, **shared})
    return in_maps


def _ensure_devices():
    import jax

    try:
        if len(jax.devices()) >= NCORES:
            return
    except Exception:
        pass
    try:
        jax.config.update("jax_platforms", "axon")
    except Exception:
        pass


def kernel(x, qkv_w, q_bias, v_bias, rpb_table, proj_w, proj_b,
           rel_pos_index, _trace=False, _trace_kwargs=None):
    _ensure_devices()
    nc = _get_program()
    in_maps = _host_prep(x, qkv_w, q_bias, v_bias, rpb_table, proj_w, proj_b,
                         rel_pos_index)
    res = run_bass_kernel_spmd(
        nc, in_maps, core_ids=list(range(NCORES)),
        trace=_trace, **(_trace_kwargs or {}),
    )
    outs = []
    for c in range(NCORES):
        oT = res.results[c]["outT"]  # [128, KT*T] fp16
        o = oT.reshape(128, KT, T).transpose(1, 0, 2).reshape(C, T)
        outs.append(np.ascontiguousarray(o.T).reshape(BC, N, C))
    out = np.concatenate(outs, axis=0).astype(np.float32)
    if _trace:
        kernel._last_results = res
    return out
